# revision 1
# baseline (speedup 1.0000x reference)
"""DDPM sampling kernel for Trainium2 — 8-core data-parallel Bass kernel.

Strategy: pure data parallelism over the 256-sample batch (32 samples/core).
The 3999-step denoising loop runs fully unrolled on-device; per core the
state z^T = (x + pe(t))^T lives in SBUF as [64 partitions x 32 samples].

Per step (t = 3999 - s), with A_t = 1/sqrt(alpha_t),
B_t = A_t * beta_t / sqrt(1 - abar_t), C_t = sqrt(beta_t):
    h1 = relu(z @ W1 + b1)        (matmul -> PSUM, DVE relu)
    h2 = relu(h1 @ W2 + b2)
    z' = A_t z - B_t (h2 @ W3) + E_s
where E_s = C_t * noise_t - B_t*b3 - A_t*pe(t) + pe(t-1)  is precomputed on
the host (noise bit-matches the reference's fold_in/normal scan) and streamed
from HBM in double-buffered chunks.  The affine update is fused into the
PSUM accumulation group of the third matmul via identity-weight matmuls.
A_t, -B_t are baked into instructions as immediates (loop is unrolled).
"""

import numpy as np

T = 4000
D = 64
HID = 128
N = 256
NCORES = 8
NS = N // NCORES
S = T - 1
BETA_START = 1e-4
BETA_END = 0.02
CH = 64  # steps per staged E chunk


def _host_schedule():
    import jax, jax.numpy as jnp
    cpu = jax.devices("cpu")[0]
    with jax.default_device(cpu):
        betas32 = np.asarray(jnp.linspace(BETA_START, BETA_END, T, dtype=jnp.float32))
        abar32 = np.asarray(jnp.cumprod(1.0 - jnp.asarray(betas32)))
    betas = betas32.astype(np.float64)
    alphas = 1.0 - betas
    abar = abar32.astype(np.float64)
    half = D // 2
    freqs = np.exp(-(np.arange(half) / half) * np.log(10000.0))
    ts = np.arange(T - 1, 0, -1)
    A = 1.0 / np.sqrt(alphas[ts])
    B = A * betas[ts] / np.sqrt(1.0 - abar[ts])
    C = np.sqrt(betas[ts])
    inp = freqs[None, :] * ts[:, None].astype(np.float64)
    PE_t = np.stack([np.sin(inp), np.cos(inp)], -1).reshape(len(ts), D)
    return A, B, C, PE_t


def _host_noise():
    import jax, jax.numpy as jnp
    cpu = jax.devices("cpu")[0]
    with jax.default_device(cpu):
        base_key = jax.random.key(42)
        ts = jnp.arange(T - 1, 0, -1)
        # lax.map (a scan) reproduces the reference's per-step
        # fold_in+normal bit-exactly; vmap would NOT.
        f = jax.jit(lambda ts: jax.lax.map(lambda t: jax.random.normal(
            jax.random.fold_in(base_key, t), (N, D), jnp.float32), ts),
            backend="cpu")
        return np.asarray(f(ts))  # [S, N, D] float32


def _build_nc(Af, Bf, with_bias):
    import concourse.mybir as mybir
    from concourse import bacc
    from concourse.tile import TileContext
    from concourse.masks import make_identity

    dt = mybir.dt.float32
    nc = bacc.Bacc()
    dW1 = nc.dram_tensor("W1", [D, HID], dt, kind="ExternalInput")
    dW2 = nc.dram_tensor("W2", [HID, HID], dt, kind="ExternalInput")
    dW3 = nc.dram_tensor("W3", [HID, D], dt, kind="ExternalInput")
    if with_bias:
        db1 = nc.dram_tensor("b1r", [1, HID], dt, kind="ExternalInput")
        db2 = nc.dram_tensor("b2r", [1, HID], dt, kind="ExternalInput")
    dE = nc.dram_tensor("E", [D, S * NS], dt, kind="ExternalInput")
    dz0 = nc.dram_tensor("z0", [D, NS], dt, kind="ExternalInput")
    dout = nc.dram_tensor("zout", [D, NS], dt, kind="ExternalOutput")

    nchunks = (S + CH - 1) // CH

    with TileContext(nc) as tc:
        with (
            tc.tile_pool(name="const", bufs=1) as const,
            tc.tile_pool(name="echunk", bufs=2) as epool,
            tc.tile_pool(name="zpool", bufs=3) as zpool,
            tc.tile_pool(name="work", bufs=2) as work,
            tc.tile_pool(name="ps1", bufs=2, space="PSUM") as ps1,
            tc.tile_pool(name="ps2", bufs=2, space="PSUM") as ps2,
            tc.tile_pool(name="ps3", bufs=2, space="PSUM") as ps3,
        ):
            w1 = const.tile([D, HID], dt)
            w2 = const.tile([HID, HID], dt)
            w3 = const.tile([HID, D], dt)
            i64 = const.tile([D, D], dt)
            nc.sync.dma_start(w1[:], dW1[:])
            nc.sync.dma_start(w2[:], dW2[:])
            nc.sync.dma_start(w3[:], dW3[:])
            make_identity(nc, i64[:])
            if with_bias:
                b1r = const.tile([1, HID], dt)
                b2r = const.tile([1, HID], dt)
                ones1 = const.tile([1, NS], dt)
                nc.sync.dma_start(b1r[:], db1[:])
                nc.sync.dma_start(b2r[:], db2[:])
                nc.gpsimd.memset(ones1[:], 1.0)

            z = zpool.tile([D, NS], dt, tag="z")
            nc.sync.dma_start(z[:], dz0[:])

            echunks = [None] * nchunks

            def stage(c):
                if c < nchunks:
                    et = epool.tile([D, min(CH, S - c * CH) * NS], dt, tag="e")
                    nc.sync.dma_start(
                        et[:], dE[:, c * CH * NS: c * CH * NS + et.shape[1]])
                    echunks[c] = et

            stage(0)
            for s in range(S):
                c, j = divmod(s, CH)
                if j == 0:
                    stage(c + 1)
                e_s = echunks[c][:, j * NS:(j + 1) * NS]
                a_s = float(Af[s])
                bn_s = float(-Bf[s])

                w3b = work.tile([HID, D], dt, tag="w3b")
                nc.vector.tensor_scalar_mul(w3b[:], w3[:], bn_s)
                az = work.tile([D, NS], dt, tag="az")
                nc.vector.tensor_scalar_mul(az[:], z[:], a_s)

                p1 = ps1.tile([HID, NS], dt, tag="p1")
                p2 = ps2.tile([HID, NS], dt, tag="p2")
                p3 = ps3.tile([D, NS], dt, tag="p3")

                if with_bias:
                    nc.tensor.matmul(p1[:], b1r[:], ones1[:], start=True, stop=False)
                    nc.tensor.matmul(p1[:], w1[:], z[:], start=False, stop=True)
                else:
                    nc.tensor.matmul(p1[:], w1[:], z[:], start=True, stop=True)
                h1 = work.tile([HID, NS], dt, tag="h1")
                nc.vector.tensor_scalar_max(h1[:], p1[:], 0.0)

                nc.tensor.matmul(p3[:], i64[:], e_s, start=True, stop=False)
                nc.tensor.matmul(p3[:], i64[:], az[:], start=False, stop=False)

                if with_bias:
                    nc.tensor.matmul(p2[:], b2r[:], ones1[:], start=True, stop=False)
                    nc.tensor.matmul(p2[:], w2[:], h1[:], start=False, stop=True)
                else:
                    nc.tensor.matmul(p2[:], w2[:], h1[:], start=True, stop=True)
                h2 = work.tile([HID, NS], dt, tag="h2")
                nc.vector.tensor_scalar_max(h2[:], p2[:], 0.0)

                nc.tensor.matmul(p3[:], w3b[:], h2[:], start=False, stop=True)
                znew = zpool.tile([D, NS], dt, tag="z")
                nc.vector.tensor_copy(znew[:], p3[:])
                z = znew

            nc.sync.dma_start(dout[:], z[:])
    nc.compile()
    return nc


def kernel(**inputs) -> np.ndarray:
    A, B, C, PE_t = _host_schedule()
    noise = _host_noise()

    x_init = np.asarray(inputs["x_init"], np.float32)
    W1 = np.ascontiguousarray(np.asarray(inputs["W1"], np.float32))
    W2 = np.ascontiguousarray(np.asarray(inputs["W2"], np.float32))
    W3 = np.ascontiguousarray(np.asarray(inputs["W3"], np.float32))
    b1 = np.asarray(inputs["b1"], np.float32)
    b2 = np.asarray(inputs["b2"], np.float32)
    b3 = np.asarray(inputs["b3"], np.float64)
    with_bias = bool(np.any(b1) or np.any(b2))

    PE_next = np.zeros((S, D))
    PE_next[:-1] = PE_t[1:]           # last step: no positional encoding added
    E = (C[:, None, None] * noise.astype(np.float64)
         + (PE_next - A[:, None] * PE_t - B[:, None] * b3[None, :])[:, None, :]
         ).astype(np.float32)          # [S, N, D]
    z0 = (x_init + PE_t[0].astype(np.float32)).astype(np.float32)

    Af = A.astype(np.float32)
    Bf = B.astype(np.float32)
    nc = _build_nc(Af, Bf, with_bias)

    shared = {"W1": W1, "W2": W2, "W3": W3}
    if with_bias:
        shared["b1r"] = b1.reshape(1, HID).copy()
        shared["b2r"] = b2.reshape(1, HID).copy()
    in_maps = []
    for c in range(NCORES):
        lo, hi = c * NS, (c + 1) * NS
        Ec = np.ascontiguousarray(E[:, lo:hi, :].transpose(2, 0, 1).reshape(D, S * NS))
        z0c = np.ascontiguousarray(z0[lo:hi].T)
        in_maps.append({**shared, "E": Ec, "z0": z0c})

    from concourse.bass_utils import run_bass_kernel_spmd
    res = run_bass_kernel_spmd(nc, in_maps, core_ids=list(range(NCORES)))
    out = np.empty((N, D), np.float32)
    for c in range(NCORES):
        out[c * NS:(c + 1) * NS] = res.results[c]["zout"].T
    return out
